# revision 6
# baseline (speedup 1.0000x reference)
"""Multi-head attention (N=4, S=T=2048, E=1024, H=16) on 8 trn2 NeuronCores.

Sharding: 8 cores = 4 batches x 2 head-groups (Megatron-style over heads).
Each core computes 8 heads of one batch and a partial output projection;
the host sums the two partials per batch and adds the output bias.

Self-contained: builds a Bass/Tile kernel and runs it via
run_bass_kernel_spmd on cores 0-7.
"""
import os
import sys

for _p in ("/opt/trn_rl_repo", "/root/.axon_site/_ro/trn_rl_repo"):
    if os.path.isdir(_p) and _p not in sys.path:
        sys.path.append(_p)

import numpy as np
import ml_dtypes  # noqa: F401  (bf16 numpy dtype availability)

import concourse.bass as bass
import concourse.mybir as mybir
import concourse.tile as tile
from concourse import bacc
from concourse.bass_utils import run_bass_kernel_spmd

F32 = mybir.dt.float32
F32R = mybir.dt.float32r
BF16 = mybir.dt.bfloat16
AF = mybir.ActivationFunctionType

# Problem constants (full problem; _build_nc is parameterized for testing)
E_FULL = 1024
H_FULL = 16
HD = 64
N_FULL, S_FULL, T_FULL = 4, 2048, 2048
N_CORES = 8

# Set by test harness to collect a profile; kernel() stores results here.
TRACE = False
TRACE_KW = {}
LAST_RESULT = [None]


def _build_nc(E, S, T, NH, SCHUNK, use_f32r=True, av_bf16=True):
    """Build the per-core kernel.

    E: model dim; S: query rows this core handles; T: kv rows; NH: heads
    on this core (head dim fixed 64); SCHUNK: s-tile for attention phase
    (multiple of 512). DG = NH*64 must be a multiple of 128.
    """
    DG = NH * HD          # head-group width on this core
    EB = E // 128         # contraction subtiles for the input projections
    DB = DG // 128        # d-blocks of Q^T / K^T; also i-blocks of Y^T
    TB = T // 128         # t-blocks
    NSC = S // SCHUNK     # attention s-chunks
    NN = SCHUNK // 512    # 512-wide matmul slices per s-chunk
    VW = NH * (HD + 1)    # V tile width: 64 dims + ones column per head
    assert DG % 128 == 0 and SCHUNK % 512 == 0 and S % SCHUNK == 0

    MMD = F32R if use_f32r else F32

    av_dt = BF16 if av_bf16 else F32

    nc = bacc.Bacc(None)
    xqT = nc.dram_tensor("xqT", [E, S], MMD, kind="ExternalInput")
    xkT = nc.dram_tensor("xkT", [E, T], MMD, kind="ExternalInput")
    xvT = nc.dram_tensor("xvT", [E, T], MMD, kind="ExternalInput")
    wqT = nc.dram_tensor("wqT", [E, DG], MMD, kind="ExternalInput")
    wkT = nc.dram_tensor("wkT", [E, DG], MMD, kind="ExternalInput")
    wvT = nc.dram_tensor("wvT", [E, DG], MMD, kind="ExternalInput")
    wpT = nc.dram_tensor("wpT", [DG, E], MMD, kind="ExternalInput")
    bq = nc.dram_tensor("bq", [DG], F32, kind="ExternalInput")
    bk = nc.dram_tensor("bk", [DG], F32, kind="ExternalInput")
    bv = nc.dram_tensor("bv", [DG], F32, kind="ExternalInput")
    out = nc.dram_tensor("out", [S, E], F32, kind="ExternalOutput")

    with tile.TileContext(nc) as tc:
        with (
            tc.tile_pool(name="const", bufs=1) as cpool,
            tc.tile_pool(name="persist", bufs=1) as ppool,
        ):
            # Biases laid out for per-partition scalar ops.
            bq_sb = cpool.tile([128, DB], F32, tag="bq")
            bk_sb = cpool.tile([128, DB], F32, tag="bk")
            bv_sb = cpool.tile([HD, NH], F32, tag="bv")
            nc.sync.dma_start(bq_sb[:], bq.rearrange("(db p) -> p db", p=128))
            nc.sync.dma_start(bk_sb[:], bk.rearrange("(db p) -> p db", p=128))
            nc.sync.dma_start(bv_sb[:], bv.rearrange("(h p) -> p h", p=HD))
            wp_sb = cpool.tile([128, DB, E], MMD, tag="wp")
            nc.sync.dma_start(wp_sb[:], wpT.rearrange("(db p) e -> p db e", p=128))

            qt_sb = ppool.tile([128, DB, S], MMD, tag="qt")   # Q^T  [d, s]
            kt_sb = ppool.tile([128, DB, T], MMD, tag="kt")   # K^T  [d, t]
            v_sb = ppool.tile([128, TB, VW], av_dt, tag="v")  # V    [t, d]+ones

            # ---------------- Phase 1: projections ----------------
            with (
                tc.tile_pool(name="ph1w", bufs=1) as wpool,
                tc.tile_pool(name="ph1s", bufs=1) as spool,
                tc.tile_pool(name="ph1p", bufs=4, space="PSUM") as qpsum,
            ):
                wq_sb = wpool.tile([128, EB, DG], MMD, tag="wq")
                wk_sb = wpool.tile([128, EB, DG], MMD, tag="wk")
                wv_sb = wpool.tile([128, EB, DG], MMD, tag="wv")
                nc.sync.dma_start(wq_sb[:], wqT.rearrange("(eb p) d -> p eb d", p=128))
                nc.sync.dma_start(wk_sb[:], wkT.rearrange("(eb p) d -> p eb d", p=128))
                nc.sync.dma_start(wv_sb[:], wvT.rearrange("(eb p) d -> p eb d", p=128))

                # Q^T[d,s] = (Wq^T)^T @ x_q^T ; K^T likewise
                for src, w_sb, b_sb, o_sb, L in (
                    (xqT, wq_sb, bq_sb, qt_sb, S),
                    (xkT, wk_sb, bk_sb, kt_sb, T),
                ):
                    for sc in range(L // 512):
                        xs = spool.tile([128, EB, 512], MMD, tag="xs")
                        nc.sync.dma_start(
                            xs[:],
                            src.rearrange("(eb p) s -> p eb s", p=128)[
                                :, :, sc * 512 : (sc + 1) * 512
                            ],
                        )
                        for db in range(DB):
                            ps = qpsum.tile([128, 512], F32, tag="pp")
                            for kb in range(EB):
                                nc.tensor.matmul(
                                    ps[:],
                                    (w_sb[:, kb, db * 128 : (db + 1) * 128]),
                                    (xs[:, kb, :]),
                                    start=(kb == 0),
                                    stop=(kb == EB - 1),
                                )
                            nc.vector.tensor_scalar_add(
                                o_sb[:, db, sc * 512 : (sc + 1) * 512],
                                ps[:],
                                b_sb[:, db : db + 1],
                            )

                # V[t,d] = (x_v^T)^T @ Wv^T   (bias deferred to Y via softmax)
                for tb in range(TB):
                    xs = spool.tile([128, EB, 128], MMD, tag="xv")
                    nc.sync.dma_start(
                        xs[:],
                        xvT.rearrange("(eb p) t -> p eb t", p=128)[
                            :, :, tb * 128 : (tb + 1) * 128
                        ],
                    )
                    VCH = min(512, DG)
                    for dc in range(DG // VCH):
                        ps = qpsum.tile([128, VCH], F32, tag="pp")
                        for kb in range(EB):
                            nc.tensor.matmul(
                                ps[:],
                                (xs[:, kb, :]),
                                (wv_sb[:, kb, dc * VCH : (dc + 1) * VCH]),
                                start=(kb == 0),
                                stop=(kb == EB - 1),
                            )
                        # scatter head dims into the ones-augmented layout
                        nhc = VCH // HD  # heads in this chunk
                        h0 = dc * nhc
                        nc.vector.tensor_copy(
                            v_sb[:, tb, h0 * (HD + 1) : (h0 + nhc) * (HD + 1)]
                            .rearrange("p (h w) -> p h w", w=HD + 1)[:, :, 0:HD],
                            ps[:].rearrange("p (h w) -> p h w", w=HD),
                        )
                    nc.vector.memset(
                        v_sb[:, tb, :].rearrange("p (h w) -> p h w", w=HD + 1)[
                            :, :, HD : HD + 1
                        ],
                        1.0,
                    )

            # ---------------- Phase 2: attention + out-proj ----------------
            with (
                tc.tile_pool(name="epool", bufs=4) as epool,
                tc.tile_pool(name="ypool", bufs=2) as ypool,
                tc.tile_pool(name="npool", bufs=2) as npool,
                tc.tile_pool(name="opool", bufs=3) as opool,
                tc.tile_pool(name="stp", bufs=2, space="PSUM") as stp,
                tc.tile_pool(name="ytp", bufs=1, space="PSUM") as ytp,
                tc.tile_pool(name="opp", bufs=2, space="PSUM") as opp,
            ):
                for sc in range(NSC):
                    yt_sb = ypool.tile([128, DB, SCHUNK], MMD, tag="yt")
                    s0 = sc * SCHUNK
                    for h in range(NH):
                        db, rh = h // 2, (h % 2) * 64
                        yt = ytp.tile([65, SCHUNK], F32, tag="ytp")
                        for tb in range(TB):
                            # scores^T[t, s] = K^T_h^T @ Q^T_h ; exp via ACT
                            st = stp.tile([128, SCHUNK], F32, tag="st")
                            for nn in range(NN):
                                nc.tensor.matmul(
                                    st[:, nn * 512 : (nn + 1) * 512],
                                    (kt_sb[rh : rh + 64, db, tb * 128 : (tb + 1) * 128]),
                                    (qt_sb[rh : rh + 64, db, s0 + nn * 512 : s0 + (nn + 1) * 512]),
                                    start=True,
                                    stop=True,
                                )
                            e_tb = epool.tile([128, SCHUNK], av_dt, tag="e")
                            nc.scalar.activation(
                                e_tb[:], st[:], AF.Exp, scale=0.125
                            )
                            # Y^T_aug[65, s] += V_aug^T @ E_tb (row 64 = sums)
                            for nn in range(NN):
                                nc.tensor.matmul(
                                    yt[:, nn * 512 : (nn + 1) * 512],
                                    v_sb[:, tb, h * (HD + 1) : (h + 1) * (HD + 1)],
                                    e_tb[:, nn * 512 : (nn + 1) * 512],
                                    start=(tb == 0),
                                    stop=(tb == TB - 1),
                                )
                        # normalize: recip of sums, broadcast down 64 rows
                        rs = npool.tile([1, SCHUNK], F32, tag="rs")
                        rbc = npool.tile([64, SCHUNK], F32, tag="rbc")
                        nc.vector.reciprocal(rs[:], yt[64:65, :])
                        nc.gpsimd.partition_broadcast(rbc[:], rs[:])
                        if rh == 0:
                            dst = yt_sb[0:64, db, :]
                            nc.vector.tensor_tensor(
                                dst, yt[0:64, :], rbc[:], mybir.AluOpType.mult
                            )
                            nc.vector.tensor_scalar_add(
                                dst, dst, bv_sb[:, h : h + 1]
                            )
                        else:
                            tmp = npool.tile([64, SCHUNK], MMD, tag="ytmp")
                            nc.vector.tensor_tensor(
                                tmp[:], yt[0:64, :], rbc[:], mybir.AluOpType.mult
                            )
                            nc.vector.tensor_scalar_add(
                                tmp[:], tmp[:], bv_sb[:, h : h + 1]
                            )
                            # partition shift 0-63 -> 64-127 (DMA only)
                            nc.sync.dma_start(yt_sb[64:128, db, :], tmp[:])

                    # out[s, :] += Y^T^T @ Wp^T  (partial; host adds pair + bias)
                    for sb in range(SCHUNK // 128):
                        for jc in range(E // 512):
                            op = opp.tile([128, 512], F32, tag="op")
                            for ib in range(DB):
                                nc.tensor.matmul(
                                    op[:],
                                    (yt_sb[:, ib, sb * 128 : (sb + 1) * 128]),
                                    (wp_sb[:, ib, jc * 512 : (jc + 1) * 512]),
                                    start=(ib == 0),
                                    stop=(ib == DB - 1),
                                )
                            ob = opool.tile([128, 512], F32, tag="ob")
                            nc.vector.tensor_copy(ob[:], op[:])
                            nc.sync.dma_start(
                                out[
                                    s0 + sb * 128 : s0 + (sb + 1) * 128,
                                    jc * 512 : (jc + 1) * 512,
                                ],
                                ob[:],
                            )

    nc.compile()
    return nc


_NC_CACHE = {}


def _get_nc(key, builder):
    if key not in _NC_CACHE:
        _NC_CACHE[key] = builder()
    return _NC_CACHE[key]


def kernel(query, key, value, Wq, bq, Wk, bk, Wv, bv, Wp, bp):
    query = np.asarray(query, np.float32)
    key = np.asarray(key, np.float32)
    value = np.asarray(value, np.float32)
    Wq, bq = np.asarray(Wq, np.float32), np.asarray(bq, np.float32)
    Wk, bk = np.asarray(Wk, np.float32), np.asarray(bk, np.float32)
    Wv, bv = np.asarray(Wv, np.float32), np.asarray(bv, np.float32)
    Wp, bp = np.asarray(Wp, np.float32), np.asarray(bp, np.float32)

    n, s, e = query.shape
    t = value.shape[1]
    assert (n, s, t, e) == (N_FULL, S_FULL, T_FULL, E_FULL)

    nc = _get_nc(
        "full",
        lambda: _build_nc(E_FULL, S_FULL, T_FULL, H_FULL // 2, 1024),
    )

    DG = (H_FULL // 2) * HD
    in_maps = []
    for c in range(N_CORES):
        b, g = c // 2, c % 2
        gs = slice(g * DG, (g + 1) * DG)
        in_maps.append(
            {
                "xqT": np.ascontiguousarray(query[b].T),
                "xkT": np.ascontiguousarray(key[b].T),
                "xvT": np.ascontiguousarray(value[b].T),
                "wqT": np.ascontiguousarray(Wq[gs, :].T),
                "wkT": np.ascontiguousarray(Wk[gs, :].T),
                "wvT": np.ascontiguousarray(Wv[gs, :].T),
                "wpT": np.ascontiguousarray(Wp[:, gs].T),
                "bq": np.ascontiguousarray(bq[gs]),
                "bk": np.ascontiguousarray(bk[gs]),
                "bv": np.ascontiguousarray(bv[gs]),
            }
        )

    res = run_bass_kernel_spmd(
        nc, in_maps, list(range(N_CORES)), trace=TRACE, **TRACE_KW
    )
    LAST_RESULT[0] = res

    outp = np.empty((n, s, e), np.float32)
    for b in range(n):
        outp[b] = res.results[2 * b]["out"] + res.results[2 * b + 1]["out"] + bp
    return outp


# revision 10
# speedup vs baseline: 1.3804x; 1.3804x over previous
"""Multi-head attention (N=4, S=T=2048, E=1024, H=16) on 8 trn2 NeuronCores.

Sharding: 8 cores = 4 batches x 2 head-groups (Megatron-style over heads).
Each core computes 8 heads of one batch and a partial output projection;
the host sums the two partials per batch and adds the output bias.

Self-contained: builds a Bass/Tile kernel and runs it via
run_bass_kernel_spmd on cores 0-7.
"""
import os
import sys

for _p in ("/opt/trn_rl_repo", "/root/.axon_site/_ro/trn_rl_repo"):
    if os.path.isdir(_p) and _p not in sys.path:
        sys.path.append(_p)

import numpy as np
import ml_dtypes  # noqa: F401  (bf16 numpy dtype availability)

import concourse.bass as bass
import concourse.mybir as mybir
import concourse.tile as tile
from concourse import bacc
from concourse.bass_utils import run_bass_kernel_spmd

F32 = mybir.dt.float32
F32R = mybir.dt.float32r
BF16 = mybir.dt.bfloat16
AF = mybir.ActivationFunctionType

# Problem constants (full problem; _build_nc is parameterized for testing)
E_FULL = 1024
H_FULL = 16
HD = 64
N_FULL, S_FULL, T_FULL = 4, 2048, 2048
N_CORES = 8

# Set by test harness to collect a profile; kernel() stores results here.
TRACE = False
TRACE_KW = {}
LAST_RESULT = [None]


def _build_nc(E, S, T, NH, SCHUNK, use_f32r=True, av_bf16=True):
    """Build the per-core kernel.

    E: model dim; S: query rows this core handles; T: kv rows; NH: heads
    on this core (head dim fixed 64); SCHUNK: s-tile for attention phase
    (multiple of 512). DG = NH*64 must be a multiple of 128.
    """
    DG = NH * HD          # head-group width on this core
    EB = E // 128         # contraction subtiles for the input projections
    DB = DG // 128        # d-blocks of Q^T / K^T; also i-blocks of Y^T
    TB = T // 128         # t-blocks
    NSC = S // SCHUNK     # attention s-chunks
    NN = SCHUNK // 512    # 512-wide matmul slices per s-chunk
    VW = NH * (HD + 1)    # V tile width: 64 dims + ones column per head
    assert DG % 128 == 0 and SCHUNK % 512 == 0 and S % SCHUNK == 0

    MMD = F32R if use_f32r else F32

    av_dt = BF16 if av_bf16 else F32

    nc = bacc.Bacc(None)
    xqT = nc.dram_tensor("xqT", [E, S], MMD, kind="ExternalInput")
    xkT = nc.dram_tensor("xkT", [E, T], MMD, kind="ExternalInput")
    xvT = nc.dram_tensor("xvT", [E, T], MMD, kind="ExternalInput")
    wqT = nc.dram_tensor("wqT", [E, DG], MMD, kind="ExternalInput")
    wkT = nc.dram_tensor("wkT", [E, DG], MMD, kind="ExternalInput")
    wvT = nc.dram_tensor("wvT", [E, DG], MMD, kind="ExternalInput")
    wpT = nc.dram_tensor("wpT", [DG, E], MMD, kind="ExternalInput")
    bq = nc.dram_tensor("bq", [DG], F32, kind="ExternalInput")
    bk = nc.dram_tensor("bk", [DG], F32, kind="ExternalInput")
    bv = nc.dram_tensor("bv", [DG], F32, kind="ExternalInput")
    out = nc.dram_tensor("out", [S, E], F32, kind="ExternalOutput")

    with tile.TileContext(nc) as tc:
        with (
            tc.tile_pool(name="const", bufs=1) as cpool,
            tc.tile_pool(name="persist", bufs=1) as ppool,
        ):
            # Biases laid out for per-partition scalar ops.
            bq_sb = cpool.tile([128, DB], F32, tag="bq")
            bk_sb = cpool.tile([128, DB], F32, tag="bk")
            bv_sb = cpool.tile([HD, NH], F32, tag="bv")
            nc.sync.dma_start(bq_sb[:], bq.rearrange("(db p) -> p db", p=128))
            nc.sync.dma_start(bk_sb[:], bk.rearrange("(db p) -> p db", p=128))
            nc.sync.dma_start(bv_sb[:], bv.rearrange("(h p) -> p h", p=HD))
            wp_sb = cpool.tile([128, DB, E], MMD, tag="wp")
            nc.sync.dma_start(wp_sb[:], wpT.rearrange("(db p) e -> p db e", p=128))

            qt_sb = ppool.tile([128, DB, S], MMD, tag="qt")   # Q^T  [d, s]
            kt_sb = ppool.tile([128, DB, T], MMD, tag="kt")   # K^T  [d, t]
            v_sb = ppool.tile([128, TB, VW], av_dt, tag="v")  # V    [t, d]+ones

            # ---------------- Phase 1: projections ----------------
            with (
                tc.tile_pool(name="ph1w", bufs=1) as wpool,
                tc.tile_pool(name="ph1s", bufs=2) as spool,
                tc.tile_pool(name="ph1p", bufs=4, space="PSUM") as qpsum,
            ):
                # Q^T[d,s] = (Wq^T)^T @ x_q^T ; K^T likewise
                for src, wdram, b_sb, o_sb, L, wtag in (
                    (xqT, wqT, bq_sb, qt_sb, S, "wq"),
                    (xkT, wkT, bk_sb, kt_sb, T, "wk"),
                ):
                    w_sb = wpool.tile([128, EB, DG], MMD, tag=wtag)
                    nc.sync.dma_start(
                        w_sb[:], wdram.rearrange("(eb p) d -> p eb d", p=128)
                    )
                    for sc in range(L // 512):
                        xs = spool.tile([128, EB, 512], MMD, tag="xs")
                        nc.sync.dma_start(
                            xs[:],
                            src.rearrange("(eb p) s -> p eb s", p=128)[
                                :, :, sc * 512 : (sc + 1) * 512
                            ],
                        )
                        for db in range(DB):
                            ps = qpsum.tile([128, 512], F32, tag="pp")
                            for kb in range(EB):
                                nc.tensor.matmul(
                                    ps[:],
                                    (w_sb[:, kb, db * 128 : (db + 1) * 128]),
                                    (xs[:, kb, :]),
                                    start=(kb == 0),
                                    stop=(kb == EB - 1),
                                )
                            nc.vector.tensor_scalar_add(
                                o_sb[:, db, sc * 512 : (sc + 1) * 512],
                                ps[:],
                                b_sb[:, db : db + 1],
                            )

                # V[t,d] = (x_v^T)^T @ Wv^T   (bias deferred to Y via softmax)
                wv_sb = wpool.tile([128, EB, DG], MMD, tag="wv")
                nc.sync.dma_start(
                    wv_sb[:], wvT.rearrange("(eb p) d -> p eb d", p=128)
                )
                for tb in range(TB):
                    xs = spool.tile([128, EB, 128], MMD, tag="xv")
                    nc.sync.dma_start(
                        xs[:],
                        xvT.rearrange("(eb p) t -> p eb t", p=128)[
                            :, :, tb * 128 : (tb + 1) * 128
                        ],
                    )
                    VCH = min(512, DG)
                    for dc in range(DG // VCH):
                        ps = qpsum.tile([128, VCH], F32, tag="pp")
                        for kb in range(EB):
                            nc.tensor.matmul(
                                ps[:],
                                (xs[:, kb, :]),
                                (wv_sb[:, kb, dc * VCH : (dc + 1) * VCH]),
                                start=(kb == 0),
                                stop=(kb == EB - 1),
                            )
                        # scatter head dims into the ones-augmented layout
                        nhc = VCH // HD  # heads in this chunk
                        h0 = dc * nhc
                        nc.vector.tensor_copy(
                            v_sb[:, tb, h0 * (HD + 1) : (h0 + nhc) * (HD + 1)]
                            .rearrange("p (h w) -> p h w", w=HD + 1)[:, :, 0:HD],
                            ps[:].rearrange("p (h w) -> p h w", w=HD),
                        )
                    nc.vector.memset(
                        v_sb[:, tb, :].rearrange("p (h w) -> p h w", w=HD + 1)[
                            :, :, HD : HD + 1
                        ],
                        1.0,
                    )

            # ---------------- Phase 2: attention + out-proj ----------------
            with (
                tc.tile_pool(name="epool", bufs=4) as epool,
                tc.tile_pool(name="ypool", bufs=1) as ypool,
                tc.tile_pool(name="npool", bufs=2) as npool,
                tc.tile_pool(name="opool", bufs=3) as opool,
                tc.tile_pool(name="stp", bufs=2, space="PSUM") as stp,
                tc.tile_pool(name="ytp", bufs=2, space="PSUM") as ytp,
                tc.tile_pool(name="opp", bufs=2, space="PSUM") as opp,
            ):
                SP = SCHUNK // 128  # spread width for the reciprocal
                for sc in range(NSC):
                    yt_sb = ypool.tile([128, DB, SCHUNK], MMD, tag="yt")
                    s0 = sc * SCHUNK
                    for hp in range(NH // 2):
                        h0, h1 = 2 * hp, 2 * hp + 1
                        db = hp
                        # SBUF staging for the two heads' Y^T_aug
                        ye = [
                            npool.tile([65, SCHUNK], F32, tag="ye0", name="ye0"),
                            npool.tile([65, SCHUNK], F32, tag="ye1", name="ye1"),
                        ]
                        for nn in range(NN):
                            n0 = nn * 512
                            yt0 = ytp.tile([65, 512], F32, tag="ytp")
                            yt1 = ytp.tile([65, 512], F32, tag="ytp")
                            for tb in range(TB):
                                # scores^T for both heads of the pair, packed
                                # into the PE array as two K=64 row-tiles.
                                st = stp.tile([128, 1024], F32, tag="st")
                                nc.tensor.matmul(
                                    st[:, 0:512],
                                    kt_sb[0:64, db, tb * 128 : (tb + 1) * 128],
                                    qt_sb[0:64, db, s0 + n0 : s0 + n0 + 512],
                                    start=True,
                                    stop=True,
                                    tile_position=(0, 0),
                                )
                                nc.tensor.matmul(
                                    st[:, 512:1024],
                                    kt_sb[64:128, db, tb * 128 : (tb + 1) * 128],
                                    qt_sb[64:128, db, s0 + n0 : s0 + n0 + 512],
                                    start=True,
                                    stop=True,
                                    tile_position=(64, 0),
                                )
                                e_tb = epool.tile([128, 1024], av_dt, tag="e")
                                nc.scalar.activation(
                                    e_tb[:], st[:], AF.Exp, scale=0.125
                                )
                                # Y^T_aug[65, s] += V_aug^T @ E  (row 64 = sums)
                                nc.tensor.matmul(
                                    yt0[:],
                                    v_sb[:, tb, h0 * (HD + 1) : (h0 + 1) * (HD + 1)],
                                    e_tb[:, 0:512],
                                    start=(tb == 0),
                                    stop=(tb == TB - 1),
                                )
                                nc.tensor.matmul(
                                    yt1[:],
                                    v_sb[:, tb, h1 * (HD + 1) : (h1 + 1) * (HD + 1)],
                                    e_tb[:, 512:1024],
                                    start=(tb == 0),
                                    stop=(tb == TB - 1),
                                )
                            # evacuate PSUM fast so the next chains can start
                            nc.vector.tensor_copy(ye[0][:, n0 : n0 + 512], yt0[:])
                            nc.vector.tensor_copy(ye[1][:, n0 : n0 + 512], yt1[:])
                        # normalize both heads: recip of sums on a partition-
                        # spread layout, broadcast, multiply, add V-bias.
                        for hi, h in ((0, h0), (1, h1)):
                            rh = (h % 2) * 64
                            rs = npool.tile([1, SCHUNK], F32, tag="rs")
                            nc.vector.reciprocal(rs[:], ye[hi][64:65, :])
                            rbc = npool.tile([64, SCHUNK], F32, tag="rbc")
                            nc.gpsimd.partition_broadcast(rbc[:], rs[:])
                            if rh == 0:
                                dst = yt_sb[0:64, db, :]
                                nc.vector.tensor_tensor(
                                    dst, ye[hi][0:64, :], rbc[:], mybir.AluOpType.mult
                                )
                                nc.vector.tensor_scalar_add(
                                    dst, dst, bv_sb[:, h : h + 1]
                                )
                            else:
                                tmp = npool.tile([64, SCHUNK], MMD, tag="ytmp")
                                nc.vector.tensor_tensor(
                                    tmp[:], ye[hi][0:64, :], rbc[:], mybir.AluOpType.mult
                                )
                                nc.vector.tensor_scalar_add(
                                    tmp[:], tmp[:], bv_sb[:, h : h + 1]
                                )
                                # partition shift 0-63 -> 64-127 (DMA only)
                                nc.sync.dma_start(yt_sb[64:128, db, :], tmp[:])

                    # out[s, :] += Y^T^T @ Wp^T  (partial; host adds pair + bias)
                    for sb in range(SCHUNK // 128):
                        for jc in range(E // 512):
                            op = opp.tile([128, 512], F32, tag="op")
                            for ib in range(DB):
                                nc.tensor.matmul(
                                    op[:],
                                    (yt_sb[:, ib, sb * 128 : (sb + 1) * 128]),
                                    (wp_sb[:, ib, jc * 512 : (jc + 1) * 512]),
                                    start=(ib == 0),
                                    stop=(ib == DB - 1),
                                )
                            ob = opool.tile([128, 512], F32, tag="ob")
                            nc.vector.tensor_copy(ob[:], op[:])
                            nc.sync.dma_start(
                                out[
                                    s0 + sb * 128 : s0 + (sb + 1) * 128,
                                    jc * 512 : (jc + 1) * 512,
                                ],
                                ob[:],
                            )

    nc.compile()
    return nc


_NC_CACHE = {}


def _get_nc(key, builder):
    if key not in _NC_CACHE:
        _NC_CACHE[key] = builder()
    return _NC_CACHE[key]


def kernel(query, key, value, Wq, bq, Wk, bk, Wv, bv, Wp, bp):
    query = np.asarray(query, np.float32)
    key = np.asarray(key, np.float32)
    value = np.asarray(value, np.float32)
    Wq, bq = np.asarray(Wq, np.float32), np.asarray(bq, np.float32)
    Wk, bk = np.asarray(Wk, np.float32), np.asarray(bk, np.float32)
    Wv, bv = np.asarray(Wv, np.float32), np.asarray(bv, np.float32)
    Wp, bp = np.asarray(Wp, np.float32), np.asarray(bp, np.float32)

    n, s, e = query.shape
    t = value.shape[1]
    assert (n, s, t, e) == (N_FULL, S_FULL, T_FULL, E_FULL)

    nc = _get_nc(
        "full",
        lambda: _build_nc(E_FULL, S_FULL, T_FULL, H_FULL // 2, 1024),
    )

    DG = (H_FULL // 2) * HD
    in_maps = []
    for c in range(N_CORES):
        b, g = c // 2, c % 2
        gs = slice(g * DG, (g + 1) * DG)
        in_maps.append(
            {
                "xqT": np.ascontiguousarray(query[b].T),
                "xkT": np.ascontiguousarray(key[b].T),
                "xvT": np.ascontiguousarray(value[b].T),
                "wqT": np.ascontiguousarray(Wq[gs, :].T),
                "wkT": np.ascontiguousarray(Wk[gs, :].T),
                "wvT": np.ascontiguousarray(Wv[gs, :].T),
                "wpT": np.ascontiguousarray(Wp[:, gs].T),
                "bq": np.ascontiguousarray(bq[gs]),
                "bk": np.ascontiguousarray(bk[gs]),
                "bv": np.ascontiguousarray(bv[gs]),
            }
        )

    res = run_bass_kernel_spmd(
        nc, in_maps, list(range(N_CORES)), trace=TRACE, **TRACE_KW
    )
    LAST_RESULT[0] = res

    outp = np.empty((n, s, e), np.float32)
    for b in range(n):
        outp[b] = res.results[2 * b]["out"] + res.results[2 * b + 1]["out"] + bp
    return outp


# revision 11
# speedup vs baseline: 1.6812x; 1.2179x over previous
"""Multi-head attention (N=4, S=T=2048, E=1024, H=16) on 8 trn2 NeuronCores.

Sharding: 8 cores = 4 batches x 2 head-groups (Megatron-style over heads).
Each core computes 8 heads of one batch and a partial output projection;
the host sums the two partials per batch and adds the output bias.

Self-contained: builds a Bass/Tile kernel and runs it via
run_bass_kernel_spmd on cores 0-7.
"""
import os
import sys

for _p in ("/opt/trn_rl_repo", "/root/.axon_site/_ro/trn_rl_repo"):
    if os.path.isdir(_p) and _p not in sys.path:
        sys.path.append(_p)

import numpy as np
import ml_dtypes  # noqa: F401  (bf16 numpy dtype availability)

import concourse.bass as bass
import concourse.mybir as mybir
import concourse.tile as tile
from concourse import bacc
from concourse.bass_utils import run_bass_kernel_spmd

F32 = mybir.dt.float32
F32R = mybir.dt.float32r
BF16 = mybir.dt.bfloat16
AF = mybir.ActivationFunctionType

# Problem constants (full problem; _build_nc is parameterized for testing)
E_FULL = 1024
H_FULL = 16
HD = 64
N_FULL, S_FULL, T_FULL = 4, 2048, 2048
N_CORES = 8

# Set by test harness to collect a profile; kernel() stores results here.
TRACE = False
TRACE_KW = {}
LAST_RESULT = [None]


def _build_nc(E, S, T, NH, SCHUNK, use_f32r=True, av_bf16=True):
    """Build the per-core kernel.

    E: model dim; S: query rows this core handles; T: kv rows; NH: heads
    on this core (head dim fixed 64); SCHUNK: s-tile for attention phase
    (multiple of 512). DG = NH*64 must be a multiple of 128.
    """
    DG = NH * HD          # head-group width on this core
    EB = E // 128         # contraction subtiles for the input projections
    DB = DG // 128        # d-blocks of Q^T / K^T; also i-blocks of Y^T
    TB = T // 128         # t-blocks
    NSC = S // SCHUNK     # attention s-chunks
    NN = SCHUNK // 512    # 512-wide matmul slices per s-chunk
    VW = NH * (HD + 1)    # V tile width: 64 dims + ones column per head
    assert DG % 128 == 0 and SCHUNK % 512 == 0 and S % SCHUNK == 0

    MMD = F32R if use_f32r else F32

    av_dt = BF16 if av_bf16 else F32

    nc = bacc.Bacc(None)
    xqT = nc.dram_tensor("xqT", [E, S], MMD, kind="ExternalInput")
    xkT = nc.dram_tensor("xkT", [E, T], MMD, kind="ExternalInput")
    xvT = nc.dram_tensor("xvT", [E, T], MMD, kind="ExternalInput")
    wqT = nc.dram_tensor("wqT", [E, DG], MMD, kind="ExternalInput")
    wkT = nc.dram_tensor("wkT", [E, DG], MMD, kind="ExternalInput")
    wvT = nc.dram_tensor("wvT", [E, DG], MMD, kind="ExternalInput")
    wpT = nc.dram_tensor("wpT", [DG, E], MMD, kind="ExternalInput")
    bq = nc.dram_tensor("bq", [DG], F32, kind="ExternalInput")
    bk = nc.dram_tensor("bk", [DG], F32, kind="ExternalInput")
    bv = nc.dram_tensor("bv", [DG], F32, kind="ExternalInput")
    out = nc.dram_tensor("out", [S, E], F32, kind="ExternalOutput")

    with tile.TileContext(nc) as tc:
        with (
            tc.tile_pool(name="const", bufs=1) as cpool,
            tc.tile_pool(name="persist", bufs=1) as ppool,
        ):
            # Biases laid out for per-partition scalar ops.
            bq_sb = cpool.tile([128, DB], F32, tag="bq")
            bk_sb = cpool.tile([128, DB], F32, tag="bk")
            bv_sb = cpool.tile([HD, NH], F32, tag="bv")
            nc.sync.dma_start(bq_sb[:], bq.rearrange("(db p) -> p db", p=128))
            nc.sync.dma_start(bk_sb[:], bk.rearrange("(db p) -> p db", p=128))
            nc.sync.dma_start(bv_sb[:], bv.rearrange("(h p) -> p h", p=HD))
            wp_sb = cpool.tile([128, DB, E], MMD, tag="wp")
            nc.sync.dma_start(wp_sb[:], wpT.rearrange("(db p) e -> p db e", p=128))

            qt_sb = ppool.tile([128, DB, S], MMD, tag="qt")   # Q^T  [d, s]
            kt_sb = ppool.tile([128, DB, T], MMD, tag="kt")   # K^T  [d, t]
            v_sb = ppool.tile([128, TB, VW], av_dt, tag="v")  # V    [t, d]+ones

            # ---------------- Phase 1: projections ----------------
            with (
                tc.tile_pool(name="ph1w", bufs=1) as wpool,
                tc.tile_pool(name="ph1s", bufs=2) as spool,
                tc.tile_pool(name="ph1p", bufs=4, space="PSUM") as qpsum,
            ):
                # Q^T[d,s] = (Wq^T)^T @ x_q^T ; K^T likewise
                for src, wdram, b_sb, o_sb, L, wtag in (
                    (xqT, wqT, bq_sb, qt_sb, S, "wq"),
                    (xkT, wkT, bk_sb, kt_sb, T, "wk"),
                ):
                    w_sb = wpool.tile([128, EB, DG], MMD, tag=wtag)
                    nc.sync.dma_start(
                        w_sb[:], wdram.rearrange("(eb p) d -> p eb d", p=128)
                    )
                    for sc in range(L // 512):
                        xs = spool.tile([128, EB, 512], MMD, tag="xs")
                        nc.sync.dma_start(
                            xs[:],
                            src.rearrange("(eb p) s -> p eb s", p=128)[
                                :, :, sc * 512 : (sc + 1) * 512
                            ],
                        )
                        for db in range(DB):
                            ps = qpsum.tile([128, 512], F32, tag="pp")
                            for kb in range(EB):
                                nc.tensor.matmul(
                                    ps[:],
                                    (w_sb[:, kb, db * 128 : (db + 1) * 128]),
                                    (xs[:, kb, :]),
                                    start=(kb == 0),
                                    stop=(kb == EB - 1),
                                )
                            nc.vector.tensor_scalar_add(
                                o_sb[:, db, sc * 512 : (sc + 1) * 512],
                                ps[:],
                                b_sb[:, db : db + 1],
                            )

                # V[t,d] = (x_v^T)^T @ Wv^T   (bias deferred to Y via softmax)
                wv_sb = wpool.tile([128, EB, DG], MMD, tag="wv")
                nc.sync.dma_start(
                    wv_sb[:], wvT.rearrange("(eb p) d -> p eb d", p=128)
                )
                for tb in range(TB):
                    xs = spool.tile([128, EB, 128], MMD, tag="xv")
                    nc.sync.dma_start(
                        xs[:],
                        xvT.rearrange("(eb p) t -> p eb t", p=128)[
                            :, :, tb * 128 : (tb + 1) * 128
                        ],
                    )
                    VCH = min(512, DG)
                    for dc in range(DG // VCH):
                        ps = qpsum.tile([128, VCH], F32, tag="pp")
                        for kb in range(EB):
                            nc.tensor.matmul(
                                ps[:],
                                (xs[:, kb, :]),
                                (wv_sb[:, kb, dc * VCH : (dc + 1) * VCH]),
                                start=(kb == 0),
                                stop=(kb == EB - 1),
                            )
                        # scatter head dims into the ones-augmented layout
                        nhc = VCH // HD  # heads in this chunk
                        h0 = dc * nhc
                        nc.vector.tensor_copy(
                            v_sb[:, tb, h0 * (HD + 1) : (h0 + nhc) * (HD + 1)]
                            .rearrange("p (h w) -> p h w", w=HD + 1)[:, :, 0:HD],
                            ps[:].rearrange("p (h w) -> p h w", w=HD),
                        )
                    nc.vector.memset(
                        v_sb[:, tb, :].rearrange("p (h w) -> p h w", w=HD + 1)[
                            :, :, HD : HD + 1
                        ],
                        1.0,
                    )

            # ---------------- Phase 2: attention + out-proj ----------------
            with (
                tc.tile_pool(name="epool", bufs=4) as epool,
                tc.tile_pool(name="ypool", bufs=1) as ypool,
                tc.tile_pool(name="npool", bufs=2) as npool,
                tc.tile_pool(name="opool", bufs=3) as opool,
                tc.tile_pool(name="stp", bufs=2, space="PSUM") as stp,
                tc.tile_pool(name="ytp", bufs=2, space="PSUM") as ytp,
                tc.tile_pool(name="opp", bufs=2, space="PSUM") as opp,
            ):
                SP = SCHUNK // 128  # spread width for the reciprocal
                for sc in range(NSC):
                    yt_sb = ypool.tile([128, DB, SCHUNK], MMD, tag="yt")
                    s0 = sc * SCHUNK

                    def normalize(ye, hpair):
                        # recip of sums on a partition-spread layout (cheap),
                        # broadcast down 64 rows, multiply, add V-bias.
                        for hi, h in ((0, hpair[0]), (1, hpair[1])):
                            db_, rh = h // 2, (h % 2) * 64
                            sp = npool.tile([128, SP], F32, tag="sp", name="sp")
                            nc.sync.dma_start(sp[:], ye[hi][64:65, :])
                            nc.vector.reciprocal(sp[:], sp[:])
                            rs = npool.tile([1, SCHUNK], F32, tag="rs", name="rs")
                            nc.sync.dma_start(rs[:], sp[:])
                            rbc = npool.tile([64, SCHUNK], F32, tag="rbc", name="rbc")
                            nc.gpsimd.partition_broadcast(rbc[:], rs[:])
                            if rh == 0:
                                dst = yt_sb[0:64, db_, :]
                                nc.vector.tensor_tensor(
                                    dst, ye[hi][0:64, :], rbc[:], mybir.AluOpType.mult
                                )
                                nc.vector.tensor_scalar_add(
                                    dst, dst, bv_sb[:, h : h + 1]
                                )
                            else:
                                tmp = npool.tile([64, SCHUNK], MMD, tag="ytmp", name="tmp")
                                nc.vector.tensor_tensor(
                                    tmp[:], ye[hi][0:64, :], rbc[:], mybir.AluOpType.mult
                                )
                                nc.vector.tensor_scalar_add(
                                    tmp[:], tmp[:], bv_sb[:, h : h + 1]
                                )
                                # partition shift 0-63 -> 64-127 (DMA only)
                                nc.sync.dma_start(yt_sb[64:128, db_, :], tmp[:])

                    pend = None
                    for hp in range(NH // 2):
                        h0, h1 = 2 * hp, 2 * hp + 1
                        db = hp
                        # SBUF staging for the two heads' Y^T_aug
                        ye = [
                            npool.tile([65, SCHUNK], F32, tag="ye0", name="ye0"),
                            npool.tile([65, SCHUNK], F32, tag="ye1", name="ye1"),
                        ]
                        for nn in range(NN):
                            n0 = nn * 512
                            yt0 = ytp.tile([65, 512], F32, tag="ytp")
                            yt1 = ytp.tile([65, 512], F32, tag="ytp")
                            for tb in range(TB):
                                # scores^T for both heads of the pair, packed
                                # into the PE array as two K=64 row-tiles.
                                st = stp.tile([128, 1024], F32, tag="st")
                                nc.tensor.matmul(
                                    st[:, 0:512],
                                    kt_sb[0:64, db, tb * 128 : (tb + 1) * 128],
                                    qt_sb[0:64, db, s0 + n0 : s0 + n0 + 512],
                                    start=True,
                                    stop=True,
                                    tile_position=(0, 0),
                                )
                                nc.tensor.matmul(
                                    st[:, 512:1024],
                                    kt_sb[64:128, db, tb * 128 : (tb + 1) * 128],
                                    qt_sb[64:128, db, s0 + n0 : s0 + n0 + 512],
                                    start=True,
                                    stop=True,
                                    tile_position=(64, 0),
                                )
                                e_tb = epool.tile([128, 1024], av_dt, tag="e")
                                nc.scalar.activation(
                                    e_tb[:], st[:], AF.Exp, scale=0.125
                                )
                                # Y^T_aug[65, s] += V_aug^T @ E  (row 64 = sums)
                                nc.tensor.matmul(
                                    yt0[:],
                                    v_sb[:, tb, h0 * (HD + 1) : (h0 + 1) * (HD + 1)],
                                    e_tb[:, 0:512],
                                    start=(tb == 0),
                                    stop=(tb == TB - 1),
                                )
                                nc.tensor.matmul(
                                    yt1[:],
                                    v_sb[:, tb, h1 * (HD + 1) : (h1 + 1) * (HD + 1)],
                                    e_tb[:, 512:1024],
                                    start=(tb == 0),
                                    stop=(tb == TB - 1),
                                )
                            # evacuate PSUM fast so the next chains can start
                            nc.vector.tensor_copy(ye[0][:, n0 : n0 + 512], yt0[:])
                            nc.vector.tensor_copy(ye[1][:, n0 : n0 + 512], yt1[:])
                        # defer this pair's normalize until after the next
                        # pair's evacuations so the DVE queue never blocks
                        # the PSUM accumulators.
                        if pend is not None:
                            normalize(*pend)
                        pend = (ye, (h0, h1))
                    normalize(*pend)

                    # out[s, :] += Y^T^T @ Wp^T  (partial; host adds pair + bias)
                    for sb in range(SCHUNK // 128):
                        for jc in range(E // 512):
                            op = opp.tile([128, 512], F32, tag="op")
                            for ib in range(DB):
                                nc.tensor.matmul(
                                    op[:],
                                    (yt_sb[:, ib, sb * 128 : (sb + 1) * 128]),
                                    (wp_sb[:, ib, jc * 512 : (jc + 1) * 512]),
                                    start=(ib == 0),
                                    stop=(ib == DB - 1),
                                )
                            ob = opool.tile([128, 512], F32, tag="ob")
                            nc.vector.tensor_copy(ob[:], op[:])
                            nc.sync.dma_start(
                                out[
                                    s0 + sb * 128 : s0 + (sb + 1) * 128,
                                    jc * 512 : (jc + 1) * 512,
                                ],
                                ob[:],
                            )

    nc.compile()
    return nc


_NC_CACHE = {}


def _get_nc(key, builder):
    if key not in _NC_CACHE:
        _NC_CACHE[key] = builder()
    return _NC_CACHE[key]


def kernel(query, key, value, Wq, bq, Wk, bk, Wv, bv, Wp, bp):
    query = np.asarray(query, np.float32)
    key = np.asarray(key, np.float32)
    value = np.asarray(value, np.float32)
    Wq, bq = np.asarray(Wq, np.float32), np.asarray(bq, np.float32)
    Wk, bk = np.asarray(Wk, np.float32), np.asarray(bk, np.float32)
    Wv, bv = np.asarray(Wv, np.float32), np.asarray(bv, np.float32)
    Wp, bp = np.asarray(Wp, np.float32), np.asarray(bp, np.float32)

    n, s, e = query.shape
    t = value.shape[1]
    assert (n, s, t, e) == (N_FULL, S_FULL, T_FULL, E_FULL)

    nc = _get_nc(
        "full",
        lambda: _build_nc(E_FULL, S_FULL, T_FULL, H_FULL // 2, 1024),
    )

    DG = (H_FULL // 2) * HD
    in_maps = []
    for c in range(N_CORES):
        b, g = c // 2, c % 2
        gs = slice(g * DG, (g + 1) * DG)
        in_maps.append(
            {
                "xqT": np.ascontiguousarray(query[b].T),
                "xkT": np.ascontiguousarray(key[b].T),
                "xvT": np.ascontiguousarray(value[b].T),
                "wqT": np.ascontiguousarray(Wq[gs, :].T),
                "wkT": np.ascontiguousarray(Wk[gs, :].T),
                "wvT": np.ascontiguousarray(Wv[gs, :].T),
                "wpT": np.ascontiguousarray(Wp[:, gs].T),
                "bq": np.ascontiguousarray(bq[gs]),
                "bk": np.ascontiguousarray(bk[gs]),
                "bv": np.ascontiguousarray(bv[gs]),
            }
        )

    res = run_bass_kernel_spmd(
        nc, in_maps, list(range(N_CORES)), trace=TRACE, **TRACE_KW
    )
    LAST_RESULT[0] = res

    outp = np.empty((n, s, e), np.float32)
    for b in range(n):
        outp[b] = res.results[2 * b]["out"] + res.results[2 * b + 1]["out"] + bp
    return outp


# revision 12
# speedup vs baseline: 1.8247x; 1.0854x over previous
"""Multi-head attention (N=4, S=T=2048, E=1024, H=16) on 8 trn2 NeuronCores.

Sharding: 8 cores = 4 batches x 2 head-groups (Megatron-style over heads).
Each core computes 8 heads of one batch and a partial output projection;
the host sums the two partials per batch and adds the output bias.

Self-contained: builds a Bass/Tile kernel and runs it via
run_bass_kernel_spmd on cores 0-7.
"""
import os
import sys

for _p in ("/opt/trn_rl_repo", "/root/.axon_site/_ro/trn_rl_repo"):
    if os.path.isdir(_p) and _p not in sys.path:
        sys.path.append(_p)

import numpy as np
import ml_dtypes

import concourse.bass as bass
import concourse.mybir as mybir
import concourse.tile as tile
from concourse import bacc
from concourse.bass_utils import run_bass_kernel_spmd

F32 = mybir.dt.float32
F32R = mybir.dt.float32r
BF16 = mybir.dt.bfloat16
AF = mybir.ActivationFunctionType

E_FULL = 1024
H_FULL = 16
HD = 64
N_FULL, S_FULL, T_FULL = 4, 2048, 2048
N_CORES = 8

# Set by the test harness to collect a profile.
TRACE = False
TRACE_KW = {}
LAST_RESULT = [None]


def _build_nc(E, S, T, NH, SCHUNK, use_f32r=True, x_bf16=True):
    """Build the per-core kernel.

    E: model dim; S: query rows; T: kv rows; NH: heads on this core
    (head dim 64); SCHUNK: attention s-tile (multiple of 512).
    DG = NH*64 must be a multiple of 128.
    """
    DG = NH * HD
    EB = E // 128
    DB = DG // 128
    TB = T // 128
    NSC = S // SCHUNK
    NN = SCHUNK // 512
    VW = NH * (HD + 1)
    SP = SCHUNK // 128
    assert DG % 128 == 0 and SCHUNK % 512 == 0 and S % SCHUNK == 0

    MMD = F32R if use_f32r else F32   # out-projection operand dtype
    XD = BF16 if x_bf16 else MMD      # activations/projection dtype

    nc = bacc.Bacc(None)
    xqT = nc.dram_tensor("xqT", [E, S], XD, kind="ExternalInput")
    xkT = nc.dram_tensor("xkT", [E, T], XD, kind="ExternalInput")
    xvT = nc.dram_tensor("xvT", [E, T], XD, kind="ExternalInput")
    wqT = nc.dram_tensor("wqT", [E, DG], XD, kind="ExternalInput")
    wkT = nc.dram_tensor("wkT", [E, DG], XD, kind="ExternalInput")
    wvT = nc.dram_tensor("wvT", [E, DG], XD, kind="ExternalInput")
    wpT = nc.dram_tensor("wpT", [DG, E], MMD, kind="ExternalInput")
    bq = nc.dram_tensor("bq", [DG], F32, kind="ExternalInput")
    bk = nc.dram_tensor("bk", [DG], F32, kind="ExternalInput")
    bv = nc.dram_tensor("bv", [DG], F32, kind="ExternalInput")
    out = nc.dram_tensor("out", [S, E], F32, kind="ExternalOutput")

    with tile.TileContext(nc) as tc:
        with (
            tc.tile_pool(name="const", bufs=1) as cpool,
            tc.tile_pool(name="persist", bufs=1) as ppool,
            tc.tile_pool(name="stp", bufs=2, space="PSUM") as stp,
            tc.tile_pool(name="ytp", bufs=2, space="PSUM") as ytp,
            tc.tile_pool(name="opp", bufs=2, space="PSUM") as opp,
        ):
            bq_sb = cpool.tile([128, DB], F32, tag="bq")
            bk_sb = cpool.tile([128, DB], F32, tag="bk")
            bv_sb = cpool.tile([HD, NH], F32, tag="bv")
            nc.sync.dma_start(bq_sb[:], bq.rearrange("(db p) -> p db", p=128))
            nc.sync.dma_start(bk_sb[:], bk.rearrange("(db p) -> p db", p=128))
            nc.sync.dma_start(bv_sb[:], bv.rearrange("(h p) -> p h", p=HD))
            wp_sb = cpool.tile([128, DB, E], MMD, tag="wp")
            nc.sync.dma_start(wp_sb[:], wpT.rearrange("(db p) e -> p db e", p=128))

            qt_sb = ppool.tile([128, DB, S], XD, tag="qt")    # Q^T [d, s]
            kt_sb = ppool.tile([128, DB, T], XD, tag="kt")    # K^T [d, t]
            v_sb = ppool.tile([128, TB, VW], BF16, tag="v")   # V [t, d]+ones

            # ---------------- Phase 1: projections (V, K, Q) ----------------
            with (
                tc.tile_pool(name="ph1w", bufs=1) as wpool,
                tc.tile_pool(name="ph1s", bufs=2) as spool,
            ):
                # V[t,d] = (x_v^T)^T @ Wv^T  (bias deferred to Y via softmax)
                wv_sb = wpool.tile([128, EB, DG], XD, tag="wv", name="wv")
                nc.sync.dma_start(
                    wv_sb[:], wvT.rearrange("(eb p) d -> p eb d", p=128)
                )
                VCH = min(512, DG)
                for tb in range(TB):
                    xv_s = spool.tile([128, EB, 128], XD, tag="xv", name="xv")
                    nc.sync.dma_start(
                        xv_s[:],
                        xvT.rearrange("(eb p) t -> p eb t", p=128)[
                            :, :, tb * 128 : (tb + 1) * 128
                        ],
                    )
                    for dc in range(DG // VCH):
                        ps = stp.tile([128, VCH], F32, tag="st", name="psv")
                        for kb in range(EB):
                            nc.tensor.matmul(
                                ps[:],
                                xv_s[:, kb, :],
                                wv_sb[:, kb, dc * VCH : (dc + 1) * VCH],
                                start=(kb == 0),
                                stop=(kb == EB - 1),
                            )
                        nhc = VCH // HD
                        h0 = dc * nhc
                        nc.vector.tensor_copy(
                            v_sb[:, tb, h0 * (HD + 1) : (h0 + nhc) * (HD + 1)]
                            .rearrange("p (h w) -> p h w", w=HD + 1)[:, :, 0:HD],
                            ps[:].rearrange("p (h w) -> p h w", w=HD),
                        )
                    nc.vector.memset(
                        v_sb[:, tb, :].rearrange("p (h w) -> p h w", w=HD + 1)[
                            :, :, HD : HD + 1
                        ],
                        1.0,
                    )

                # K^T[d,t] then Q^T[d,s]
                for src, wdram, b_sb, o_sb, L, wtag in (
                    (xkT, wkT, bk_sb, kt_sb, T, "wk"),
                    (xqT, wqT, bq_sb, qt_sb, S, "wq"),
                ):
                    w_sb = wpool.tile([128, EB, DG], XD, tag=wtag, name=wtag)
                    nc.sync.dma_start(
                        w_sb[:], wdram.rearrange("(eb p) d -> p eb d", p=128)
                    )
                    for pc in range(L // 512):
                        xs = spool.tile([128, EB, 512], XD, tag="xs", name="xs")
                        nc.sync.dma_start(
                            xs[:],
                            src.rearrange("(eb p) s -> p eb s", p=128)[
                                :, :, pc * 512 : (pc + 1) * 512
                            ],
                        )
                        for db in range(DB):
                            ps = stp.tile([128, 512], F32, tag="st", name="psq")
                            for kb in range(EB):
                                nc.tensor.matmul(
                                    ps[:],
                                    w_sb[:, kb, db * 128 : (db + 1) * 128],
                                    xs[:, kb, :],
                                    start=(kb == 0),
                                    stop=(kb == EB - 1),
                                )
                            nc.vector.tensor_scalar_add(
                                o_sb[:, db, pc * 512 : (pc + 1) * 512],
                                ps[:],
                                b_sb[:, db : db + 1],
                            )

            # ---------------- Phase 2: attention + out-proj ----------------
            with (
                tc.tile_pool(name="epool", bufs=3) as epool,
                tc.tile_pool(name="ypool", bufs=1) as ypool,
                tc.tile_pool(name="npool", bufs=2) as npool,
                tc.tile_pool(name="n1pool", bufs=1) as n1pool,
                tc.tile_pool(name="opool", bufs=2) as opool,
            ):
                for sc in range(NSC):
                    yt_sb = ypool.tile([128, DB, SCHUNK], MMD, tag="yt")
                    s0 = sc * SCHUNK

                    def normalize(ye, hpair):
                        # 1/sums on a partition-spread layout (cheap recip),
                        # broadcast down 64 rows, multiply, add V-bias.
                        for hi, h in ((0, hpair[0]), (1, hpair[1])):
                            db_, rh = h // 2, (h % 2) * 64
                            sp = n1pool.tile([128, SP], F32, tag="sp", name="sp")
                            nc.sync.dma_start(sp[:], ye[hi][64:65, :])
                            nc.vector.reciprocal(sp[:], sp[:])
                            rs = n1pool.tile([1, SCHUNK], F32, tag="rs", name="rs")
                            nc.sync.dma_start(rs[:], sp[:])
                            rbc = n1pool.tile([64, SCHUNK], F32, tag="rbc", name="rbc")
                            nc.gpsimd.partition_broadcast(rbc[:], rs[:])
                            if rh == 0:
                                dst = yt_sb[0:64, db_, :]
                                nc.vector.tensor_tensor(
                                    dst, ye[hi][0:64, :], rbc[:], mybir.AluOpType.mult
                                )
                                nc.vector.tensor_scalar_add(
                                    dst, dst, bv_sb[:, h : h + 1]
                                )
                            else:
                                tmp = n1pool.tile(
                                    [64, SCHUNK], MMD, tag="ytmp", name="tmp"
                                )
                                nc.vector.tensor_tensor(
                                    tmp[:], ye[hi][0:64, :], rbc[:], mybir.AluOpType.mult
                                )
                                nc.vector.tensor_scalar_add(
                                    tmp[:], tmp[:], bv_sb[:, h : h + 1]
                                )
                                # partition shift 0-63 -> 64-127 (DMA only)
                                nc.sync.dma_start(yt_sb[64:128, db_, :], tmp[:])

                    pend = None
                    for hp in range(NH // 2):
                        h0, h1 = 2 * hp, 2 * hp + 1
                        db = hp
                        ye = [
                            npool.tile([65, SCHUNK], F32, tag="ye0", name="ye0"),
                            npool.tile([65, SCHUNK], F32, tag="ye1", name="ye1"),
                        ]
                        for nn in range(NN):
                            n0 = nn * 512
                            yt0 = ytp.tile([65, 512], F32, tag="ytp", name="yt0")
                            yt1 = ytp.tile([65, 512], F32, tag="ytp", name="yt1")
                            for tb in range(TB):
                                # scores^T for both heads, packed as two
                                # K=64 row-tiles of the PE array.
                                st = stp.tile([128, 1024], F32, tag="st", name="st")
                                nc.tensor.matmul(
                                    st[:, 0:512],
                                    kt_sb[0:64, db, tb * 128 : (tb + 1) * 128],
                                    qt_sb[0:64, db, s0 + n0 : s0 + n0 + 512],
                                    start=True,
                                    stop=True,
                                    tile_position=(0, 0),
                                )
                                nc.tensor.matmul(
                                    st[:, 512:1024],
                                    kt_sb[64:128, db, tb * 128 : (tb + 1) * 128],
                                    qt_sb[64:128, db, s0 + n0 : s0 + n0 + 512],
                                    start=True,
                                    stop=True,
                                    tile_position=(64, 0),
                                )
                                e_tb = epool.tile([128, 1024], BF16, tag="e")
                                nc.scalar.activation(
                                    e_tb[:], st[:], AF.Exp, scale=0.125
                                )
                                # Y^T_aug[65, s] += V_aug^T @ E (row 64 = sums)
                                nc.tensor.matmul(
                                    yt0[:],
                                    v_sb[:, tb, h0 * (HD + 1) : (h0 + 1) * (HD + 1)],
                                    e_tb[:, 0:512],
                                    start=(tb == 0),
                                    stop=(tb == TB - 1),
                                )
                                nc.tensor.matmul(
                                    yt1[:],
                                    v_sb[:, tb, h1 * (HD + 1) : (h1 + 1) * (HD + 1)],
                                    e_tb[:, 512:1024],
                                    start=(tb == 0),
                                    stop=(tb == TB - 1),
                                )
                            # evacuate PSUM fast so the next chains can start
                            nc.vector.tensor_copy(ye[0][:, n0 : n0 + 512], yt0[:])
                            nc.vector.tensor_copy(ye[1][:, n0 : n0 + 512], yt1[:])
                        # normalize is deferred one pair so the DVE queue
                        # never delays PSUM evacuation.
                        if pend is not None:
                            normalize(*pend)
                        pend = (ye, (h0, h1))
                    normalize(*pend)

                    # out[s,:] partial = Y^T^T @ Wp^T (host adds pair + bias)
                    for sb in range(SCHUNK // 128):
                        for jc in range(E // 512):
                            op = opp.tile([128, 512], F32, tag="op", name="op")
                            for ib in range(DB):
                                nc.tensor.matmul(
                                    op[:],
                                    yt_sb[:, ib, sb * 128 : (sb + 1) * 128],
                                    wp_sb[:, ib, jc * 512 : (jc + 1) * 512],
                                    start=(ib == 0),
                                    stop=(ib == DB - 1),
                                )
                            ob = opool.tile([128, 512], F32, tag="ob")
                            nc.vector.tensor_copy(ob[:], op[:])
                            nc.sync.dma_start(
                                out[
                                    s0 + sb * 128 : s0 + (sb + 1) * 128,
                                    jc * 512 : (jc + 1) * 512,
                                ],
                                ob[:],
                            )

    nc.compile()
    return nc


_NC_CACHE = {}


def _get_nc(key, builder):
    if key not in _NC_CACHE:
        _NC_CACHE[key] = builder()
    return _NC_CACHE[key]


def kernel(query, key, value, Wq, bq, Wk, bk, Wv, bv, Wp, bp):
    query = np.asarray(query, np.float32)
    key = np.asarray(key, np.float32)
    value = np.asarray(value, np.float32)
    Wq, bq = np.asarray(Wq, np.float32), np.asarray(bq, np.float32)
    Wk, bk = np.asarray(Wk, np.float32), np.asarray(bk, np.float32)
    Wv, bv = np.asarray(Wv, np.float32), np.asarray(bv, np.float32)
    Wp, bp = np.asarray(Wp, np.float32), np.asarray(bp, np.float32)

    n, s, e = query.shape
    t = value.shape[1]
    assert (n, s, t, e) == (N_FULL, S_FULL, T_FULL, E_FULL)

    nc = _get_nc(
        "full",
        lambda: _build_nc(E_FULL, S_FULL, T_FULL, H_FULL // 2, 1024),
    )

    DG = (H_FULL // 2) * HD
    bf = ml_dtypes.bfloat16
    in_maps = []
    for c in range(N_CORES):
        b, g = c // 2, c % 2
        gs = slice(g * DG, (g + 1) * DG)
        in_maps.append(
            {
                "xqT": query[b].T.astype(bf),
                "xkT": key[b].T.astype(bf),
                "xvT": value[b].T.astype(bf),
                "wqT": Wq[gs, :].T.astype(bf),
                "wkT": Wk[gs, :].T.astype(bf),
                "wvT": Wv[gs, :].T.astype(bf),
                "wpT": np.ascontiguousarray(Wp[:, gs].T),
                "bq": np.ascontiguousarray(bq[gs]),
                "bk": np.ascontiguousarray(bk[gs]),
                "bv": np.ascontiguousarray(bv[gs]),
            }
        )

    res = run_bass_kernel_spmd(
        nc, in_maps, list(range(N_CORES)), trace=TRACE, **TRACE_KW
    )
    LAST_RESULT[0] = res

    outp = np.empty((n, s, e), np.float32)
    for b in range(n):
        outp[b] = res.results[2 * b]["out"] + res.results[2 * b + 1]["out"] + bp
    return outp
